# revision 5
# baseline (speedup 1.0000x reference)
"""Trainium2 Bass kernel for APPNP-style GNN message passing (8 NeuronCores).

Algorithm (matches the jax reference):
  v = x @ lin_w;  w_dst = 1/(deg+eps) with deg = out-edge count by e[0]
  z_0 = 0;  z_k = gamma * w_dst * segsum_{e0}(z_{k-1}[e1]) + alpha * v   (10 iters)
  out = LayerNorm(z_10 + x @ skip_w + lin_b) * ln_g + ln_b

Sharding: destination nodes split across 8 cores (T*128 padded rows each).
Each iteration: AllGather z rows -> z_full (bf16 per-core HBM replica); each
core gathers its edges' source rows via dma_gather (<=1024 int16 indices per
call, 4 table chunks), builds one-hot segment matrices on the DVE, reduces
per-dst-tile on the PE (PSUM accumulation), then applies the w / alpha*v
epilogue. The s=max|v| scaling of the reference cancels (linearity) and is
skipped.
"""
import numpy as np
import ml_dtypes
import concourse.bass as bass
import concourse.bacc as bacc
import concourse.mybir as mybir
import concourse.tile as tile
from concourse.bass_utils import run_bass_kernel_spmd
from concourse.masks import make_identity

NC = 8
D = 128
ITERS = 10
ALPHA = 0.1
GAMMA = 1.0 - ALPHA
EPS = 1e-16
LN_EPS = 1e-5
NCHUNK = 4

_cache = {}


def build(T, B):
    """T = dst tiles per core; B = 128-edge blocks per (tile, chunk) cell."""
    R = T * 128
    NF = NC * R
    assert NF % NCHUNK == 0
    CH = NF // NCHUNK
    assert CH <= 32767 and B * 128 <= 1024
    CELL = B * 128                # idx slots per (tile, chunk) cell
    nc = bacc.Bacc("TRN2", target_bir_lowering=False, num_devices=NC)
    f32 = mybir.dt.float32
    bf16 = mybir.dt.bfloat16

    x_rows = nc.dram_tensor("x_rows", [R, D], f32, kind="ExternalInput")
    idx_in = nc.dram_tensor("idx_in", [128, T * NCHUNK * (CELL // 16)],
                            mybir.dt.int16, kind="ExternalInput")
    e0_in = nc.dram_tensor("e0_in", [128, T * NCHUNK * B], bf16, kind="ExternalInput")
    wg_in = nc.dram_tensor("wg_in", [128, T], f32, kind="ExternalInput")
    lin_w = nc.dram_tensor("lin_w", [D, D], f32, kind="ExternalInput")
    skip_w = nc.dram_tensor("skip_w", [D, D], f32, kind="ExternalInput")
    lin_b = nc.dram_tensor("lin_b", [1, D], f32, kind="ExternalInput")
    ln_g = nc.dram_tensor("ln_g", [1, D], f32, kind="ExternalInput")
    ln_b = nc.dram_tensor("ln_b", [1, D], f32, kind="ExternalInput")
    out_rows = nc.dram_tensor("out_rows", [R, D], f32, kind="ExternalOutput")

    z_rows = [nc.dram_tensor(f"z_rows{j}", [R, D], bf16, kind="Internal") for j in range(2)]
    z_full = [nc.dram_tensor(f"z_full{j}", [NF, D], bf16, kind="Internal", addr_space="Shared")
              for j in range(2)]
    skip_dram = nc.dram_tensor("skip_dram", [R, D], f32, kind="Internal")
    z10_dram = nc.dram_tensor("z10_dram", [R, D], f32, kind="Internal")

    def bcast_ap(t):
        a = t[:]
        return bass.AP(tensor=a.tensor, offset=a.offset, ap=[[0, 128]] + a.ap[1:])

    with tile.TileContext(nc) as tc:
        with tc.tile_pool(name="one", bufs=1) as one, \
             tc.tile_pool(name="work", bufs=3) as work, \
             tc.tile_pool(name="gio", bufs=12) as gio, \
             tc.tile_pool(name="sgp", bufs=8) as sgp, \
             tc.tile_pool(name="stg", bufs=4) as stg, \
             tc.tile_pool(name="ps", bufs=2, space="PSUM") as ps:

            ident = one.tile([128, 128], f32)
            make_identity(nc, ident[:])
            iota_i = one.tile([128, 128], mybir.dt.int32)
            nc.gpsimd.iota(iota_i[:], pattern=[[1, 128]], base=0, channel_multiplier=0)
            iota_h = one.tile([128, 128], bf16)
            nc.vector.tensor_copy(out=iota_h[:], in_=iota_i[:])
            lw_sb = one.tile([D, D], f32)
            nc.sync.dma_start(out=lw_sb[:], in_=lin_w[:])
            sw_sb = one.tile([D, D], f32)
            nc.sync.dma_start(out=sw_sb[:], in_=skip_w[:])
            linb_bc = one.tile([128, D], f32)
            nc.sync.dma_start(out=linb_bc[:], in_=bcast_ap(lin_b))
            lng_bc = one.tile([128, D], f32)
            nc.sync.dma_start(out=lng_bc[:], in_=bcast_ap(ln_g))
            lnb_bc = one.tile([128, D], f32)
            nc.sync.dma_start(out=lnb_bc[:], in_=bcast_ap(ln_b))
            eps_t = one.tile([128, 1], f32)
            nc.vector.memset(eps_t[:], LN_EPS)
            idx_sb = one.tile([128, T * NCHUNK * (CELL // 16)], mybir.dt.int16)
            nc.sync.dma_start(out=idx_sb[:], in_=idx_in[:])
            e0_sb = one.tile([128, T * NCHUNK * B], bf16)
            nc.sync.dma_start(out=e0_sb[:], in_=e0_in[:])
            wg_sb = one.tile([128, T], f32)
            nc.sync.dma_start(out=wg_sb[:], in_=wg_in[:])
            av_sb = one.tile([128, R], f32)

            # ---- phase 0 ----
            for t in range(T):
                rs = slice(t * 128, (t + 1) * 128)
                x_t = work.tile([128, D], f32, tag="x_t")
                nc.sync.dma_start(out=x_t[:], in_=x_rows[rs, :])
                xT_ps = ps.tile([128, 128], f32, tag="xT_ps")
                nc.tensor.transpose(out=xT_ps[:], in_=x_t[:], identity=ident[:])
                xT = work.tile([128, 128], f32, tag="xT")
                nc.vector.tensor_copy(out=xT[:], in_=xT_ps[:])
                v_ps = ps.tile([128, D], f32, tag="v_ps")
                nc.tensor.matmul(out=v_ps[:], lhsT=xT[:], rhs=lw_sb[:], start=True, stop=True)
                nc.scalar.mul(out=av_sb[:, rs], in_=v_ps[:], mul=ALPHA)
                z1h = stg.tile([128, D], bf16, tag="z1h")
                nc.scalar.mul(out=z1h[:], in_=v_ps[:], mul=ALPHA)
                nc.sync.dma_start(out=z_rows[0][rs, :], in_=z1h[:])
                s_ps = ps.tile([128, D], f32, tag="s_ps")
                nc.tensor.matmul(out=s_ps[:], lhsT=xT[:], rhs=sw_sb[:], start=True, stop=True)
                s_st = stg.tile([128, D], f32, tag="s_st")
                nc.vector.tensor_add(out=s_st[:], in0=s_ps[:], in1=linb_bc[:])
                nc.sync.dma_start(out=skip_dram[rs, :], in_=s_st[:])

            # ---- iterations ----
            for k in range(2, ITERS + 1):
                src = k % 2
                dst = (k + 1) % 2
                nc.gpsimd.collective_compute(
                    "AllGather", mybir.AluOpType.bypass,
                    replica_groups=[list(range(NC))],
                    ins=[z_rows[src][:]], outs=[z_full[src][:]],
                )
                for t in range(T):
                    rs = slice(t * 128, (t + 1) * 128)
                    acc = ps.tile([128, D], f32, tag="acc")
                    for c in range(NCHUNK):
                        cell = t * NCHUNK + c
                        msg = gio.tile([128, B, D], bf16, tag="msg")
                        nc.gpsimd.dma_gather(
                            out_ap=msg[:],
                            in_ap=z_full[src][c * CH:(c + 1) * CH, :],
                            idxs_ap=idx_sb[:, cell * (CELL // 16):(cell + 1) * (CELL // 16)],
                            num_idxs=CELL, num_idxs_reg=CELL, elem_size=D)
                        seg = sgp.tile([128, B, 128], bf16, tag="seg")
                        e0a = e0_sb[:, cell * B:(cell + 1) * B]
                        e0b = bass.AP(tensor=e0a.tensor, offset=e0a.offset,
                                      ap=[e0a.ap[0], e0a.ap[1], [0, 128]])
                        ioa = iota_h[:]
                        iob = bass.AP(tensor=ioa.tensor, offset=ioa.offset,
                                      ap=[ioa.ap[0], [0, B], ioa.ap[1]])
                        nc.vector.tensor_tensor(out=seg[:], in0=e0b, in1=iob,
                                                op=mybir.AluOpType.is_equal)
                        for b in range(B):
                            nc.tensor.matmul(
                                out=acc[:], lhsT=seg[:, b, :], rhs=msg[:, b, :],
                                start=(c == 0 and b == 0),
                                stop=(c == NCHUNK - 1 and b == B - 1))
                    if k < ITERS:
                        z_st = stg.tile([128, D], bf16, tag="z_st")
                        nc.vector.scalar_tensor_tensor(
                            out=z_st[:], in0=acc[:], scalar=wg_sb[:, t:t + 1],
                            in1=av_sb[:, rs],
                            op0=mybir.AluOpType.mult, op1=mybir.AluOpType.add)
                        nc.sync.dma_start(out=z_rows[dst][rs, :], in_=z_st[:])
                    else:
                        zf_st = stg.tile([128, D], f32, tag="zf_st")
                        nc.vector.scalar_tensor_tensor(
                            out=zf_st[:], in0=acc[:], scalar=wg_sb[:, t:t + 1],
                            in1=av_sb[:, rs],
                            op0=mybir.AluOpType.mult, op1=mybir.AluOpType.add)
                        nc.sync.dma_start(out=z10_dram[rs, :], in_=zf_st[:])

            # ---- phase 2 ----
            for t in range(T):
                rs = slice(t * 128, (t + 1) * 128)
                zt = work.tile([128, D], f32, tag="zt")
                nc.sync.dma_start(out=zt[:], in_=z10_dram[rs, :])
                sk = work.tile([128, D], f32, tag="sk")
                nc.sync.dma_start(out=sk[:], in_=skip_dram[rs, :])
                nc.vector.tensor_add(out=zt[:], in0=zt[:], in1=sk[:])
                stats = work.tile([128, nc.vector.BN_STATS_DIM], f32, tag="stats")
                nc.vector.bn_stats(out=stats[:], in_=zt[:])
                mv = work.tile([128, nc.vector.BN_AGGR_DIM], f32, tag="mv")
                nc.vector.bn_aggr(out=mv[:], in_=stats[:])
                rstd = work.tile([128, 1], f32, tag="rstd")
                nc.scalar.activation(out=rstd[:], in_=mv[:, 1:2],
                                     func=mybir.ActivationFunctionType.Sqrt,
                                     bias=eps_t[:], scale=1.0)
                nc.vector.reciprocal(out=rstd[:], in_=rstd[:])
                nc.vector.tensor_scalar(
                    out=zt[:], in0=zt[:], scalar1=mv[:, 0:1], scalar2=rstd[:],
                    op0=mybir.AluOpType.subtract, op1=mybir.AluOpType.mult)
                nc.vector.tensor_mul(out=zt[:], in0=zt[:], in1=lng_bc[:])
                o_st = stg.tile([128, D], f32, tag="o_st")
                nc.vector.tensor_add(out=o_st[:], in0=zt[:], in1=lnb_bc[:])
                nc.sync.dma_start(out=out_rows[rs, :], in_=o_st[:])

    nc.finalize()
    return nc


def prepare_inputs(x, e, lin_w, lin_b, skip_w, ln_g, ln_b, T, B):
    N = x.shape[0]
    R = T * 128
    NF = NC * R
    CH = NF // NCHUNK
    CELL = B * 128
    RN = (N + NC - 1) // NC
    assert RN <= R
    dst = np.asarray(e[0], np.int64)
    src = np.asarray(e[1], np.int64)
    deg = np.bincount(dst, minlength=N).astype(np.float64)
    wg_full = (GAMMA / (deg + EPS)).astype(np.float32)

    core_of = dst // RN
    loc = dst - core_of * RN
    tile_of = loc // 128
    slot_of = loc % 128
    src_core = src // RN
    src_pad = src_core * R + (src - src_core * RN)
    chunk_of = src_pad // CH
    local_of = (src_pad % CH).astype(np.int64)

    bf = ml_dtypes.bfloat16
    in_maps = []
    for c in range(NC):
        m = core_of == c
        key = tile_of[m] * NCHUNK + chunk_of[m]
        d_slot = slot_of[m]
        s_loc = local_of[m]
        order = np.argsort(key, kind="stable")
        key, d_slot, s_loc = key[order], d_slot[order], s_loc[order]
        bounds = np.searchsorted(key, np.arange(T * NCHUNK + 1))
        counts = np.diff(bounds)
        assert counts.max(initial=0) <= CELL, f"cell overflow: {counts.max()} > {CELL}"
        j_in_cell = np.arange(key.size) - np.repeat(bounds[:-1], counts)
        gslot = key * CELL + j_in_cell
        idx16 = np.zeros(T * NCHUNK * CELL, np.int16)
        idx16[gslot] = s_loc
        e0f = np.full((128, T * NCHUNK * B), -1.0, np.float32)
        e0f[gslot % 128, key * B + j_in_cell // 128] = d_slot
        # wrap int16 indices: slot j -> partition j%16 (replicated x8), col j//16
        ncols = (T * NCHUNK * CELL) // 16
        wrapped = np.zeros((16, ncols), np.int16)
        jj = np.arange(T * NCHUNK * CELL)
        wrapped[jj % 16, jj // 16] = idx16
        idx_arr = np.tile(wrapped, (8, 1))

        xr = np.zeros((R, x.shape[1]), np.float32)
        n0, n1 = c * RN, min((c + 1) * RN, N)
        xr[: n1 - n0] = x[n0:n1]
        wpad = np.zeros(R, np.float32)
        wpad[: n1 - n0] = wg_full[n0:n1]
        in_maps.append({
            "x_rows": xr, "idx_in": idx_arr, "e0_in": e0f.astype(bf),
            "wg_in": wpad.reshape(T, 128).T.copy(),
            "lin_w": np.asarray(lin_w, np.float32),
            "skip_w": np.asarray(skip_w, np.float32),
            "lin_b": np.asarray(lin_b, np.float32).reshape(1, -1),
            "ln_g": np.asarray(ln_g, np.float32).reshape(1, -1),
            "ln_b": np.asarray(ln_b, np.float32).reshape(1, -1),
        })
    return in_maps


def run(x, e, lin_w, lin_b, skip_w, ln_g, ln_b, T, B, trace=False):
    key = (T, B)
    if key not in _cache:
        _cache[key] = build(T, B)
    nc = _cache[key]
    in_maps = prepare_inputs(x, e, lin_w, lin_b, skip_w, ln_g, ln_b, T, B)
    res = run_bass_kernel_spmd(nc, in_maps, core_ids=list(range(NC)), trace=trace)
    N = x.shape[0]
    RN = (N + NC - 1) // NC
    parts = [res.results[c]["out_rows"][: min((c + 1) * RN, N) - c * RN]
             for c in range(NC)]
    return np.concatenate(parts, axis=0), res


def _required_B(e, N, T):
    """Smallest uniform blocks-per-cell that fits every (core,tile,chunk)."""
    R = T * 128
    CH = NC * R // NCHUNK
    RN = (N + NC - 1) // NC
    dst = np.asarray(e[0], np.int64)
    src = np.asarray(e[1], np.int64)
    core_of = dst // RN
    tile_of = (dst - core_of * RN) // 128
    src_core = src // RN
    chunk_of = (src_core * R + (src - src_core * RN)) // CH
    cell = (core_of * T + tile_of) * NCHUNK + chunk_of
    mx = int(np.bincount(cell, minlength=NC * T * NCHUNK).max())
    return max(1, -(-mx // 128))


def kernel(x, e, lin_w, lin_b, skip_w, ln_g, ln_b):
    x = np.asarray(x, np.float32)
    e = np.asarray(e)
    B = max(5, _required_B(e, x.shape[0], 98))
    assert B * 128 <= 1024, f"edge distribution too skewed for dma_gather: B={B}"
    out, _ = run(x, e, lin_w, lin_b, skip_w, ln_g, ln_b, T=98, B=B)
    return out.astype(np.float32)
